# revision 27
# baseline (speedup 1.0000x reference)
"""Adaptive embedding (4-bucket) lookup + projection on 8 TRN2 NeuronCores.

Strategy: pure data-parallel over the 16384 tokens (no collectives).
  Host: bucket every token by its embedding table, deduplicate each table to
        the rows actually referenced (<= n_tokens distinct rows, so gather
        indices always fit int16), sort each bucket's tokens by row for HBM
        locality, and deal them evenly across the 8 cores so every core runs
        an identical-shape program.  Tables are pre-cast to bf16 with rows
        padded to a multiple of 128 elements; projections are pre-transposed,
        pre-scaled by sqrt(D) and zero-padded to match.
  Core: one dma_gather(transpose=True) per table pulls that bucket's
        embedding rows from HBM directly into d-on-partitions (matmul lhsT)
        layout; accumulating matmuls against the resident projT produce
        [128 tokens, 1024] in PSUM; DVE/ACT alternate evacuating to bf16 in
        SBUF; plain DMA stores the rows.
  Host: rows are scattered back to original token order and upcast to f32.
"""

import os
import sys

import numpy as np

for _p in ("/opt/trn_rl_repo",):
    if _p not in sys.path:
        sys.path.insert(0, _p)

import ml_dtypes

BF16 = ml_dtypes.bfloat16

N_TOKEN = 267735
CUTS = (0, 20000, 40000, 200000, N_TOKEN)
D_TBL = (1024, 256, 64, 16)
D_PAD = (1024, 256, 128, 128)
D_OUT = 1024
EMB_SCALE = float(D_OUT) ** 0.5
N_CORES = 8
P = 128

_PROGRAM_CACHE = {}
LAST_RESULTS = None  # BassKernelResults of the most recent run (for profiling)


def _build_program(active, slot_counts, out_counts, tbl_rows):
    """Build + compile the per-core Bass program.

    active: tuple of table ids with nonzero token count
    slot_counts / out_counts: per active table — gather slots (mult of 128)
        and output row count (identical on every core)
    tbl_rows: rows of each deduplicated bf16 table
    """
    import concourse.bacc as bacc
    import concourse.mybir as mybir
    import concourse.tile as tile

    dt = mybir.dt
    nc = bacc.Bacc("TRN2", target_bir_lowering=False, debug=False,
                   num_swdge_queues=4)

    embs = {
        t: nc.dram_tensor(f"embt{t}", [tbl_rows[t], D_PAD[t]], dt.bfloat16,
                          kind="ExternalInput")
        for t in active
    }
    projs = {
        t: nc.dram_tensor(f"projt{t}", [D_PAD[t], D_OUT], dt.bfloat16,
                          kind="ExternalInput")
        for t in active
    }
    total_slots = sum(slot_counts[t] for t in active)
    idx = nc.dram_tensor("idx", [P, total_slots // 16], dt.int16,
                         kind="ExternalInput")
    # table 0 goes through indirect_dma_start (base firmware) + PE
    # transposes so its matmuls can run while the mlp library loads
    ind0 = 0 in active and D_PAD[0] // P > 1
    if ind0:
        n_c0 = slot_counts[0] // P
        idx0t = nc.dram_tensor("idx0t", [P, n_c0], dt.int32,
                               kind="ExternalInput")
        ident = nc.dram_tensor("ident", [P, P], dt.bfloat16,
                               kind="ExternalInput")
    R = sum(out_counts[t] for t in active)
    outb = nc.dram_tensor("outb", [R, D_OUT], dt.bfloat16, kind="ExternalOutput")

    from concourse.library_config import mlp

    with tile.TileContext(nc) as tc:
        with (
            tc.tile_pool(name="const", bufs=1) as const_pool,
            tc.tile_pool(name="gath", bufs=1) as gath_pool,
            tc.tile_pool(name="evac", bufs=1) as evac_pool,
            tc.tile_pool(name="psum", bufs=3, space="PSUM") as psum_pool,
            tc.tile_pool(name="tpsum", bufs=2, space="PSUM") as tpsum_pool,
        ):
            import concourse.bass as bass

            # t0 prefix inputs land first on the sync queue
            ind_insts = []
            if ind0:
                idx32_sb = const_pool.tile([P, n_c0], dt.int32, tag="idx0t")
                nc.sync.dma_start(idx32_sb[:], idx0t[:])
                ident_sb = const_pool.tile([P, P], dt.bfloat16, tag="ident")
                nc.sync.dma_start(ident_sb[:], ident[:])
                row_sb = []
                for c in range(n_c0):
                    rt = const_pool.tile([P, D_PAD[0]], dt.bfloat16,
                                         tag=f"r0{c}")
                    ii = nc.gpsimd.indirect_dma_start(
                        out=rt[:],
                        out_offset=None,
                        in_=embs[0][:, :],
                        in_offset=bass.IndirectOffsetOnAxis(
                            ap=idx32_sb[:, c:c + 1], axis=0),
                    )
                    row_sb.append(rt)
                    ind_insts.append(ii)

            # the Q7 mlp library (dma_gather) takes ~10us to land — load it
            # right after the indirect gathers' descriptor generation (base
            # firmware must not run while the library DMA is in flight)
            lib_inst = nc.gpsimd.load_library(mlp)
            for ii in ind_insts:
                bass._add_dep_helper(lib_inst.ins, ii.ins, sync=False,
                                     reason="lib load after indirect gathers")

            # all token-index tiles in one small DMA, first in the queue
            idx_sb = const_pool.tile([P, total_slots // 16], dt.int16, tag="idx")
            nc.sync.dma_start(idx_sb[:], idx[:])

            # gathers: rows land transposed, [128, K, C] = emb^T K-tiles.
            # The Q7 gather kernel's index scratch caps num_idxs (~1K crashes
            # on HW) — split big gathers into <=MAX_GATHER column slices, and
            # spread pieces across the 4 SWDGE queues (distinct Q7 core
            # pairs) so their descriptor generation runs concurrently.
            MAX_GATHER = 768
            pieces = []  # (table, tile, col0, size, idx_off)
            gath_sb = {}
            off = 0
            for t in active:
                K = D_PAD[t] // P
                C = slot_counts[t]
                gt = gath_pool.tile([P, K, C], dt.bfloat16, tag=f"g{t}")
                gath_sb[t] = gt
                if t == 0 and ind0:
                    off += C
                    continue
                n_piece = -(-C // MAX_GATHER)
                piece = -(-(C // P) // n_piece) * P
                assert n_piece == 1 or K == 1
                for c0 in range(0, C, piece):
                    cs = min(piece, C - c0)
                    pieces.append((t, gt, c0, cs, off + c0, n_piece > 1))
                off += C
            # schedule: big pieces first, round-robin over the 4 queues.
            # NOTE: overflow gathers (beyond one per queue) must cycle back
            # to queue 0 — a second gather issued on queue 3 while others
            # are in flight corrupts lanes 4/6/7 of concurrent gathers
            # (HW-reproduced; see probe5 experiments).
            pieces.sort(key=lambda p: -p[3])
            for i, (t, gt, c0, cs, ioff, sliced) in enumerate(pieces):
                q = i % 4
                nc.gpsimd.dma_gather(
                    gt[:, :, c0:c0 + cs] if sliced else gt[:],
                    embs[t][:, :],
                    idx_sb[:, ioff // 16:(ioff + cs) // 16],
                    cs,
                    cs,
                    D_PAD[t],
                    transpose=True,
                    queue_num=q,
                )

            # transpose the indirect-gathered t0 rows into gath_sb[0]
            # ([128 tok, 1024] -> 8 x [128 d, 128 tok]) on PE while the
            # library load is still in flight
            if ind0:
                for c in range(n_c0):
                    for kt in range(D_PAD[0] // P):
                        tp = tpsum_pool.tile([P, P], dt.bfloat16, tag="tp")
                        nc.tensor.transpose(
                            tp[:], row_sb[c][:, kt * P:(kt + 1) * P],
                            ident_sb[:])
                        dst = gath_sb[0][:, kt, c * P:(c + 1) * P]
                        if kt % 2 == 0:
                            nc.vector.tensor_copy(dst, tp[:])
                        else:
                            nc.scalar.copy(dst, tp[:])

            # resident projections: [Dp, 1024] -> [128, K, 1024].
            # Split each into per-K-tile DMAs so the first matmuls only wait
            # for the K-tiles they read.
            proj_sb = {}
            for t in active:
                K = D_PAD[t] // P
                pt = const_pool.tile([P, K, D_OUT], dt.bfloat16, tag=f"proj{t}")
                src = projs[t][:, :].rearrange("(k p) n -> p k n", p=P)
                for k in range(K):
                    nc.sync.dma_start(pt[:, k, :], src[:, k, :])
                proj_sb[t] = pt

            # per 128-token chunk: accumulate over K into PSUM; as soon as
            # each 512-wide bank's chain completes, evacuate that half on
            # DVE / ACT (one engine per half, in parallel); store each
            # table with 1-2 big DMAs from a per-table staging tile
            row0 = 0
            for t in active:
                K = D_PAD[t] // P
                n_c = -(-out_counts[t] // P)
                ev = evac_pool.tile([P, n_c, D_OUT], dt.bfloat16, tag=f"ev{t}")
                for c in range(n_c):
                    ps = psum_pool.tile([P, D_OUT], dt.float32, tag="ps")
                    for n in range(2):
                        for kt in range(K):
                            nc.tensor.matmul(
                                ps[:, n * 512:(n + 1) * 512],
                                gath_sb[t][:, kt, c * P:(c + 1) * P],
                                proj_sb[t][:, kt, n * 512:(n + 1) * 512],
                                start=(kt == 0),
                                stop=(kt == K - 1),
                            )
                        half = ev[:, c, n * 512:(n + 1) * 512]
                        if n == 0:
                            nc.vector.tensor_copy(half, ps[:, :512])
                        else:
                            nc.scalar.copy(half, ps[:, 512:])
                fc, rem = divmod(out_counts[t], P)
                if fc:
                    nc.sync.dma_start(
                        outb[row0:row0 + fc * P, :]
                        .rearrange("(c p) n -> p c n", p=P),
                        ev[:, :fc, :],
                    )
                if rem:
                    nc.sync.dma_start(
                        outb[row0 + fc * P: row0 + fc * P + rem, :],
                        ev[:rem, fc, :],
                    )
                row0 += out_counts[t]

    nc.finalize()
    return nc


def _host_prep(inp):
    """Bucket tokens by table; dedup rows; sort by row; per-core counts."""
    flat = np.asarray(inp).reshape(-1).astype(np.int64)

    tbl = np.searchsorted(np.asarray(CUTS[1:]), flat, side="right")
    local = flat - np.asarray(CUTS)[tbl]

    positions = {}
    lidx = {}
    uniq = {}
    for t in range(4):
        pos = np.nonzero(tbl == t)[0]
        if not pos.size:
            continue
        rows = local[pos]
        u, inv = np.unique(rows, return_inverse=True)
        order = np.argsort(inv, kind="stable")   # sort tokens by table row
        positions[t] = pos[order]
        lidx[t] = inv[order].astype(np.int16)
        uniq[t] = u

    active = tuple(sorted(positions.keys()))
    out_counts = {}
    slot_counts = {}
    for t in active:
        n = len(positions[t])
        cg = -(-n // N_CORES)           # ceil(n / 8): rows per core
        out_counts[t] = cg
        slot_counts[t] = max(P, -(-cg // P) * P)
    return flat, active, positions, lidx, uniq, out_counts, slot_counts


def _idx_tensor(active, lidx, slot_counts, core):
    """Combined int16 [128, total_slots/16] tile for one core.

    Slot j of a group at [j%16, j//16] within the group's column window;
    pads read row 0.  HW's dma_gather on SWDGE queue q reads the indices
    from partitions 32q+16 .. 32q+31 while CoreSim reads 0-15 — write all
    five ranges so any queue assignment (and the sim) sees them.
    """
    total = sum(slot_counts[t] for t in active)
    arr = np.zeros((P, total // 16), np.int16)
    off = 0
    for t in active:
        li = lidx[t][core::N_CORES]
        j = np.arange(len(li))
        for base in (0, 16, 48, 80, 112):
            arr[base + j % 16, off // 16 + j // 16] = li
        off += slot_counts[t]
    return arr


def _prep_compact_tables(active, uniq, raw_tables, raw_projs):
    tables = {}
    projTs = {}
    for t in active:
        emb = raw_tables[t]
        sel = np.asarray(emb, dtype=np.float32)[uniq[t]]
        tb = np.zeros((len(uniq[t]), D_PAD[t]), BF16)
        tb[:, :emb.shape[1]] = sel.astype(BF16)
        tables[t] = tb
        proj = raw_projs[t]
        pt = np.zeros((D_PAD[t], D_OUT), np.float32)
        pt[:proj.shape[1], :] = (np.asarray(proj, np.float32) * EMB_SCALE).T
        projTs[t] = pt.astype(BF16)
    return tables, projTs


def kernel(inp, emb0, emb1, emb2, emb3, proj0, proj1, proj2, proj3):
    global LAST_RESULTS
    from concourse.bass_utils import run_bass_kernel_spmd

    flat, active, positions, lidx, uniq, out_counts, slot_counts = \
        _host_prep(inp)
    T = flat.shape[0]

    tables, projTs = _prep_compact_tables(
        active, uniq, (emb0, emb1, emb2, emb3), (proj0, proj1, proj2, proj3))
    tbl_rows = {t: tables[t].shape[0] for t in active}

    key = (active, tuple(slot_counts[t] for t in active),
           tuple(out_counts[t] for t in active),
           tuple(tbl_rows[t] for t in active))
    nc = _PROGRAM_CACHE.get(key)
    if nc is None:
        nc = _build_program(active, slot_counts, out_counts, tbl_rows)
        _PROGRAM_CACHE[key] = nc

    ind0 = 0 in active and D_PAD[0] // P > 1
    ident = np.eye(P, dtype=BF16)
    in_maps = []
    for k in range(N_CORES):
        m = {}
        for t in active:
            m[f"embt{t}"] = tables[t]
            m[f"projt{t}"] = projTs[t]
        m["idx"] = _idx_tensor(active, lidx, slot_counts, k)
        if ind0:
            n_c0 = slot_counts[0] // P
            li = lidx[0][k::N_CORES].astype(np.int32)
            i0 = np.zeros(n_c0 * P, np.int32)
            i0[:len(li)] = li
            m["idx0t"] = np.ascontiguousarray(
                i0.reshape(n_c0, P).T)  # [128, n_c0]
            m["ident"] = ident
        in_maps.append(m)

    trace = bool(os.environ.get("KERNEL_TRACE"))
    res = run_bass_kernel_spmd(nc, in_maps, core_ids=list(range(N_CORES)),
                               trace=trace)
    LAST_RESULTS = res

    out = np.empty((T, D_OUT), np.float32)
    bases = {}
    r0 = 0
    for t in active:
        bases[t] = r0
        r0 += out_counts[t]
    for k in range(N_CORES):
        ob = np.asarray(res.results[k]["outb"])
        for t in active:
            pos = positions[t][k::N_CORES]
            if pos.size:
                out[pos] = ob[bases[t]:bases[t] + len(pos)].astype(np.float32)

    return out.reshape(*np.asarray(inp).shape, D_OUT)


# revision 29
# speedup vs baseline: 1.0863x; 1.0863x over previous
"""Adaptive embedding (4-bucket) lookup + projection on 8 TRN2 NeuronCores.

Strategy: pure data-parallel over the 16384 tokens (no collectives).
  Host: bucket every token by its embedding table, deduplicate each table to
        the rows actually referenced (<= n_tokens distinct rows, so gather
        indices always fit int16), sort each bucket's tokens by row for HBM
        locality, and deal them evenly across the 8 cores so every core runs
        an identical-shape program.  Tables are pre-cast to bf16 with rows
        padded to a multiple of 128 elements; projections are pre-transposed,
        pre-scaled by sqrt(D) and zero-padded to match.
  Core: one dma_gather(transpose=True) per table pulls that bucket's
        embedding rows from HBM directly into d-on-partitions (matmul lhsT)
        layout; accumulating matmuls against the resident projT produce
        [128 tokens, 1024] in PSUM; DVE/ACT alternate evacuating to bf16 in
        SBUF; plain DMA stores the rows.
  Host: rows are scattered back to original token order and upcast to f32.
"""

import os
import sys

import numpy as np

for _p in ("/opt/trn_rl_repo",):
    if _p not in sys.path:
        sys.path.insert(0, _p)

import ml_dtypes

BF16 = ml_dtypes.bfloat16

N_TOKEN = 267735
CUTS = (0, 20000, 40000, 200000, N_TOKEN)
D_TBL = (1024, 256, 64, 16)
D_PAD = (1024, 256, 128, 128)
D_OUT = 1024
EMB_SCALE = float(D_OUT) ** 0.5
N_CORES = 8
P = 128

_PROGRAM_CACHE = {}
LAST_RESULTS = None  # BassKernelResults of the most recent run (for profiling)


def _build_program(active, slot_counts, out_counts, tbl_rows):
    """Build + compile the per-core Bass program.

    active: tuple of table ids with nonzero token count
    slot_counts / out_counts: per active table — gather slots (mult of 128)
        and output row count (identical on every core)
    tbl_rows: rows of each deduplicated bf16 table
    """
    import concourse.bacc as bacc
    import concourse.mybir as mybir
    import concourse.tile as tile

    dt = mybir.dt
    nc = bacc.Bacc("TRN2", target_bir_lowering=False, debug=False,
                   num_swdge_queues=4)

    embs = {
        t: nc.dram_tensor(f"embt{t}", [tbl_rows[t], D_PAD[t]], dt.bfloat16,
                          kind="ExternalInput")
        for t in active
    }
    projs = {
        t: nc.dram_tensor(f"projt{t}", [D_PAD[t], D_OUT], dt.bfloat16,
                          kind="ExternalInput")
        for t in active
    }
    total_slots = sum(slot_counts[t] for t in active)
    idx = nc.dram_tensor("idx", [P, total_slots // 16], dt.int16,
                         kind="ExternalInput")
    # table 0 goes through indirect_dma_start (base firmware) + PE
    # transposes so its matmuls can run while the mlp library loads
    ind0 = 0 in active and D_PAD[0] // P > 1
    if ind0:
        n_c0 = slot_counts[0] // P
        idx0t = nc.dram_tensor("idx0t", [P, n_c0], dt.int32,
                               kind="ExternalInput")
        ident = nc.dram_tensor("ident", [P, P], dt.bfloat16,
                               kind="ExternalInput")
    R = sum(out_counts[t] for t in active)
    outb = nc.dram_tensor("outb", [R, D_OUT], dt.bfloat16, kind="ExternalOutput")

    from concourse.library_config import mlp

    with tile.TileContext(nc) as tc:
        with (
            tc.tile_pool(name="const", bufs=1) as const_pool,
            tc.tile_pool(name="gath", bufs=1) as gath_pool,
            tc.tile_pool(name="evac", bufs=1) as evac_pool,
            tc.tile_pool(name="psum", bufs=3, space="PSUM") as psum_pool,
            tc.tile_pool(name="tpsum", bufs=2, space="PSUM") as tpsum_pool,
        ):
            import concourse.bass as bass

            # the Q7 mlp library (dma_gather) takes ~10us to land — load it
            # first; the indirect t0 gathers (base firmware) overlap it
            nc.gpsimd.load_library(mlp)

            # t0 prefix inputs land first on the sync queue
            ind_insts = []
            if ind0:
                idx32_sb = const_pool.tile([P, n_c0], dt.int32, tag="idx0t")
                nc.sync.dma_start(idx32_sb[:], idx0t[:])
                ident_sb = const_pool.tile([P, P], dt.bfloat16, tag="ident")
                nc.sync.dma_start(ident_sb[:], ident[:])
                row_sb = []
                for c in range(n_c0):
                    rt = const_pool.tile([P, D_PAD[0]], dt.bfloat16,
                                         tag=f"r0{c}")
                    ii = nc.gpsimd.indirect_dma_start(
                        out=rt[:],
                        out_offset=None,
                        in_=embs[0][:, :],
                        in_offset=bass.IndirectOffsetOnAxis(
                            ap=idx32_sb[:, c:c + 1], axis=0),
                    )
                    row_sb.append(rt)
                    ind_insts.append(ii)

            # all token-index tiles in one small DMA, first in the queue
            idx_sb = const_pool.tile([P, total_slots // 16], dt.int16, tag="idx")
            nc.sync.dma_start(idx_sb[:], idx[:])

            # gathers: rows land transposed, [128, K, C] = emb^T K-tiles.
            # The Q7 gather kernel's index scratch caps num_idxs (~1K crashes
            # on HW) — split big gathers into <=MAX_GATHER column slices, and
            # spread pieces across the 4 SWDGE queues (distinct Q7 core
            # pairs) so their descriptor generation runs concurrently.
            MAX_GATHER = 768
            pieces = []  # (table, tile, col0, size, idx_off)
            gath_sb = {}
            off = 0
            for t in active:
                K = D_PAD[t] // P
                C = slot_counts[t]
                gt = gath_pool.tile([P, K, C], dt.bfloat16, tag=f"g{t}")
                gath_sb[t] = gt
                if t == 0 and ind0:
                    off += C
                    continue
                n_piece = -(-C // MAX_GATHER)
                piece = -(-(C // P) // n_piece) * P
                assert n_piece == 1 or K == 1
                for c0 in range(0, C, piece):
                    cs = min(piece, C - c0)
                    pieces.append((t, gt, c0, cs, off + c0, n_piece > 1))
                off += C
            # schedule: big pieces first, round-robin over the 4 queues.
            # NOTE: overflow gathers (beyond one per queue) must cycle back
            # to queue 0 — a second gather issued on queue 3 while others
            # are in flight corrupts lanes 4/6/7 of concurrent gathers
            # (HW-reproduced; see probe5 experiments).
            pieces.sort(key=lambda p: -p[3])
            for i, (t, gt, c0, cs, ioff, sliced) in enumerate(pieces):
                q = i % 4
                nc.gpsimd.dma_gather(
                    gt[:, :, c0:c0 + cs] if sliced else gt[:],
                    embs[t][:, :],
                    idx_sb[:, ioff // 16:(ioff + cs) // 16],
                    cs,
                    cs,
                    D_PAD[t],
                    transpose=True,
                    queue_num=q,
                )

            # transpose the indirect-gathered t0 rows into gath_sb[0]
            # ([128 tok, 1024] -> 8 x [128 d, 128 tok]) on PE while the
            # library load is still in flight
            if ind0:
                for c in range(n_c0):
                    for kt in range(D_PAD[0] // P):
                        tp = tpsum_pool.tile([P, P], dt.bfloat16, tag="tp")
                        nc.tensor.transpose(
                            tp[:], row_sb[c][:, kt * P:(kt + 1) * P],
                            ident_sb[:])
                        dst = gath_sb[0][:, kt, c * P:(c + 1) * P]
                        if kt % 2 == 0:
                            nc.vector.tensor_copy(dst, tp[:])
                        else:
                            nc.scalar.copy(dst, tp[:])

            # resident projections: [Dp, 1024] -> [128, K, 1024].
            # Split each into per-K-tile DMAs so the first matmuls only wait
            # for the K-tiles they read.
            proj_sb = {}
            for t in active:
                K = D_PAD[t] // P
                pt = const_pool.tile([P, K, D_OUT], dt.bfloat16, tag=f"proj{t}")
                src = projs[t][:, :].rearrange("(k p) n -> p k n", p=P)
                for k in range(K):
                    nc.sync.dma_start(pt[:, k, :], src[:, k, :])
                proj_sb[t] = pt

            # per 128-token chunk: accumulate over K into PSUM; as soon as
            # each 512-wide bank's chain completes, evacuate that half on
            # DVE / ACT (one engine per half, in parallel); store each
            # table with 1-2 big DMAs from a per-table staging tile
            row0 = 0
            for t in active:
                K = D_PAD[t] // P
                n_c = -(-out_counts[t] // P)
                ev = evac_pool.tile([P, n_c, D_OUT], dt.bfloat16, tag=f"ev{t}")
                for c in range(n_c):
                    ps = psum_pool.tile([P, D_OUT], dt.float32, tag="ps")
                    for n in range(2):
                        for kt in range(K):
                            nc.tensor.matmul(
                                ps[:, n * 512:(n + 1) * 512],
                                gath_sb[t][:, kt, c * P:(c + 1) * P],
                                proj_sb[t][:, kt, n * 512:(n + 1) * 512],
                                start=(kt == 0),
                                stop=(kt == K - 1),
                            )
                        half = ev[:, c, n * 512:(n + 1) * 512]
                        if n == 0:
                            nc.vector.tensor_copy(half, ps[:, :512])
                        else:
                            nc.scalar.copy(half, ps[:, 512:])
                fc, rem = divmod(out_counts[t], P)
                if fc:
                    nc.sync.dma_start(
                        outb[row0:row0 + fc * P, :]
                        .rearrange("(c p) n -> p c n", p=P),
                        ev[:, :fc, :],
                    )
                if rem:
                    nc.sync.dma_start(
                        outb[row0 + fc * P: row0 + fc * P + rem, :],
                        ev[:rem, fc, :],
                    )
                row0 += out_counts[t]

    nc.finalize()
    return nc


def _host_prep(inp):
    """Bucket tokens by table; dedup rows; sort by row; per-core counts."""
    flat = np.asarray(inp).reshape(-1).astype(np.int64)

    tbl = np.searchsorted(np.asarray(CUTS[1:]), flat, side="right")
    local = flat - np.asarray(CUTS)[tbl]

    positions = {}
    lidx = {}
    uniq = {}
    for t in range(4):
        pos = np.nonzero(tbl == t)[0]
        if not pos.size:
            continue
        rows = local[pos]
        u, inv = np.unique(rows, return_inverse=True)
        order = np.argsort(inv, kind="stable")   # sort tokens by table row
        positions[t] = pos[order]
        lidx[t] = inv[order].astype(np.int16)
        uniq[t] = u

    active = tuple(sorted(positions.keys()))
    out_counts = {}
    slot_counts = {}
    for t in active:
        n = len(positions[t])
        cg = -(-n // N_CORES)           # ceil(n / 8): rows per core
        out_counts[t] = cg
        slot_counts[t] = max(P, -(-cg // P) * P)
    return flat, active, positions, lidx, uniq, out_counts, slot_counts


def _idx_tensor(active, lidx, slot_counts, core):
    """Combined int16 [128, total_slots/16] tile for one core.

    Slot j of a group at [j%16, j//16] within the group's column window;
    pads read row 0.  HW's dma_gather on SWDGE queue q reads the indices
    from partitions 32q+16 .. 32q+31 while CoreSim reads 0-15 — write all
    five ranges so any queue assignment (and the sim) sees them.
    """
    total = sum(slot_counts[t] for t in active)
    arr = np.zeros((P, total // 16), np.int16)
    off = 0
    for t in active:
        li = lidx[t][core::N_CORES]
        j = np.arange(len(li))
        for base in (0, 16, 48, 80, 112):
            arr[base + j % 16, off // 16 + j // 16] = li
        off += slot_counts[t]
    return arr


def _prep_compact_tables(active, uniq, raw_tables, raw_projs):
    tables = {}
    projTs = {}
    for t in active:
        emb = raw_tables[t]
        sel = np.asarray(emb, dtype=np.float32)[uniq[t]]
        tb = np.zeros((len(uniq[t]), D_PAD[t]), BF16)
        tb[:, :emb.shape[1]] = sel.astype(BF16)
        tables[t] = tb
        proj = raw_projs[t]
        pt = np.zeros((D_PAD[t], D_OUT), np.float32)
        pt[:proj.shape[1], :] = (np.asarray(proj, np.float32) * EMB_SCALE).T
        projTs[t] = pt.astype(BF16)
    return tables, projTs


def kernel(inp, emb0, emb1, emb2, emb3, proj0, proj1, proj2, proj3):
    global LAST_RESULTS
    from concourse.bass_utils import run_bass_kernel_spmd

    flat, active, positions, lidx, uniq, out_counts, slot_counts = \
        _host_prep(inp)
    T = flat.shape[0]

    tables, projTs = _prep_compact_tables(
        active, uniq, (emb0, emb1, emb2, emb3), (proj0, proj1, proj2, proj3))
    tbl_rows = {t: tables[t].shape[0] for t in active}

    key = (active, tuple(slot_counts[t] for t in active),
           tuple(out_counts[t] for t in active),
           tuple(tbl_rows[t] for t in active))
    nc = _PROGRAM_CACHE.get(key)
    if nc is None:
        nc = _build_program(active, slot_counts, out_counts, tbl_rows)
        _PROGRAM_CACHE[key] = nc

    ind0 = 0 in active and D_PAD[0] // P > 1
    ident = np.eye(P, dtype=BF16)
    in_maps = []
    for k in range(N_CORES):
        m = {}
        for t in active:
            m[f"embt{t}"] = tables[t]
            m[f"projt{t}"] = projTs[t]
        m["idx"] = _idx_tensor(active, lidx, slot_counts, k)
        if ind0:
            n_c0 = slot_counts[0] // P
            li = lidx[0][k::N_CORES].astype(np.int32)
            i0 = np.zeros(n_c0 * P, np.int32)
            i0[:len(li)] = li
            m["idx0t"] = np.ascontiguousarray(
                i0.reshape(n_c0, P).T)  # [128, n_c0]
            m["ident"] = ident
        in_maps.append(m)

    trace = bool(os.environ.get("KERNEL_TRACE"))
    res = run_bass_kernel_spmd(nc, in_maps, core_ids=list(range(N_CORES)),
                               trace=trace)
    LAST_RESULTS = res

    out = np.empty((T, D_OUT), np.float32)
    bases = {}
    r0 = 0
    for t in active:
        bases[t] = r0
        r0 += out_counts[t]
    for k in range(N_CORES):
        ob = np.asarray(res.results[k]["outb"])
        for t in active:
            pos = positions[t][k::N_CORES]
            if pos.size:
                out[pos] = ob[bases[t]:bases[t] + len(pos)].astype(np.float32)

    return out.reshape(*np.asarray(inp).shape, D_OUT)


# revision 33
# speedup vs baseline: 1.1175x; 1.0287x over previous
"""Adaptive embedding (4-bucket) lookup + projection on 8 TRN2 NeuronCores.

Strategy: pure data-parallel over the 16384 tokens (no collectives).
  Host: bucket every token by its embedding table, deduplicate each table to
        the rows actually referenced (<= n_tokens distinct rows, so gather
        indices always fit int16), sort each bucket's tokens by row for HBM
        locality, and deal them evenly across the 8 cores so every core runs
        an identical-shape program.  Tables are pre-cast to bf16 with rows
        padded to a multiple of 128 elements; projections are pre-transposed,
        pre-scaled by sqrt(D) and zero-padded to match.
  Core: one dma_gather(transpose=True) per table pulls that bucket's
        embedding rows from HBM directly into d-on-partitions (matmul lhsT)
        layout; accumulating matmuls against the resident projT produce
        [128 tokens, 1024] in PSUM; DVE/ACT alternate evacuating to bf16 in
        SBUF; plain DMA stores the rows.
  Host: rows are scattered back to original token order and upcast to f32.
"""

import os
import sys

import numpy as np

for _p in ("/opt/trn_rl_repo",):
    if _p not in sys.path:
        sys.path.insert(0, _p)

import ml_dtypes

BF16 = ml_dtypes.bfloat16

N_TOKEN = 267735
CUTS = (0, 20000, 40000, 200000, N_TOKEN)
D_TBL = (1024, 256, 64, 16)
D_PAD = (1024, 256, 128, 128)
D_OUT = 1024
EMB_SCALE = float(D_OUT) ** 0.5
N_CORES = 8
P = 128

_PROGRAM_CACHE = {}
LAST_RESULTS = None  # BassKernelResults of the most recent run (for profiling)


def _build_program(active, slot_counts, out_counts, tbl_rows):
    """Build + compile the per-core Bass program.

    active: tuple of table ids with nonzero token count
    slot_counts / out_counts: per active table — gather slots (mult of 128)
        and output row count (identical on every core)
    tbl_rows: rows of each deduplicated bf16 table
    """
    import concourse.bacc as bacc
    import concourse.mybir as mybir
    import concourse.tile as tile

    dt = mybir.dt
    nc = bacc.Bacc("TRN2", target_bir_lowering=False, debug=False,
                   num_swdge_queues=4)

    embs = {
        t: nc.dram_tensor(f"embt{t}", [tbl_rows[t], D_PAD[t]], dt.bfloat16,
                          kind="ExternalInput")
        for t in active
    }
    projs = {
        t: nc.dram_tensor(f"projt{t}", [D_PAD[t], D_OUT], dt.bfloat16,
                          kind="ExternalInput")
        for t in active
    }
    total_slots = sum(slot_counts[t] for t in active)
    idx = nc.dram_tensor("idx", [P, total_slots // 16], dt.int16,
                         kind="ExternalInput")
    R = sum(out_counts[t] for t in active)
    outb = nc.dram_tensor("outb", [R, D_OUT], dt.bfloat16, kind="ExternalOutput")

    from concourse.library_config import mlp

    with tile.TileContext(nc) as tc:
        with (
            tc.tile_pool(name="const", bufs=1) as const_pool,
            tc.tile_pool(name="gath", bufs=1) as gath_pool,
            tc.tile_pool(name="evac", bufs=1) as evac_pool,
            tc.tile_pool(name="psum", bufs=8, space="PSUM") as psum_pool,
        ):
            # the Q7 mlp library (dma_gather) takes ~10us to land — start the
            # load as early as possible
            nc.gpsimd.load_library(mlp)

            # all token-index tiles in one small DMA, first in the queue
            idx_sb = const_pool.tile([P, total_slots // 16], dt.int16, tag="idx")
            nc.sync.dma_start(idx_sb[:], idx[:])

            # gathers: rows land transposed, [128, K, C] = emb^T K-tiles.
            # The Q7 gather kernel's index scratch caps num_idxs (~1K crashes
            # on HW) — split big gathers into <=MAX_GATHER column slices, and
            # spread pieces across the 4 SWDGE queues (distinct Q7 core
            # pairs) so their descriptor generation runs concurrently.
            MAX_GATHER = 768
            pieces = []  # (table, tile, col0, size, idx_off)
            gath_sb = {}
            off = 0
            for t in active:
                K = D_PAD[t] // P
                C = slot_counts[t]
                gt = gath_pool.tile([P, K, C], dt.bfloat16, tag=f"g{t}")
                n_piece = -(-C // MAX_GATHER)
                piece = -(-(C // P) // n_piece) * P
                assert n_piece == 1 or K == 1
                for c0 in range(0, C, piece):
                    cs = min(piece, C - c0)
                    pieces.append((t, gt, c0, cs, off + c0, n_piece > 1))
                gath_sb[t] = gt
                off += C
            # schedule: table 0 first (its matmuls gate the PE start; the
            # first-dispatched gather begins ~2us before the rest), then big
            # pieces, round-robin over the 4 queues.
            # NOTE: overflow gathers (beyond one per queue) must cycle back
            # to queue 0 — a second gather issued on queue 3 while others
            # are in flight corrupts lanes 4/6/7 of concurrent gathers
            # (HW-reproduced; see probe5 experiments).
            pieces.sort(key=lambda p: (p[0] != 0, -p[3]))
            for i, (t, gt, c0, cs, ioff, sliced) in enumerate(pieces):
                q = i % 4
                nc.gpsimd.dma_gather(
                    gt[:, :, c0:c0 + cs] if sliced else gt[:],
                    embs[t][:, :],
                    idx_sb[:, ioff // 16:(ioff + cs) // 16],
                    cs,
                    cs,
                    D_PAD[t],
                    transpose=True,
                    queue_num=q,
                )

            # resident projections: [Dp, 1024] -> [128, K, 1024].
            # Split each into per-K-tile DMAs so the first matmuls only wait
            # for the K-tiles they read.
            proj_sb = {}
            for t in active:
                K = D_PAD[t] // P
                pt = const_pool.tile([P, K, D_OUT], dt.bfloat16, tag=f"proj{t}")
                src = projs[t][:, :].rearrange("(k p) n -> p k n", p=P)
                for k in range(K):
                    nc.sync.dma_start(pt[:, k, :], src[:, k, :])
                proj_sb[t] = pt

            # per 128-token chunk: accumulate over K into PSUM; as soon as
            # each 512-wide bank's chain completes, evacuate that half on
            # DVE / ACT (one engine per half, in parallel); store each
            # table with 1-2 big DMAs from a per-table staging tile
            row0 = 0
            for t in active:
                K = D_PAD[t] // P
                n_c = -(-out_counts[t] // P)
                ev = evac_pool.tile([P, n_c, D_OUT], dt.bfloat16, tag=f"ev{t}")
                for c in range(n_c):
                    for n in range(2):
                        ps = psum_pool.tile([P, 512], dt.float32, tag="ps")
                        for kt in range(K):
                            nc.tensor.matmul(
                                ps[:],
                                gath_sb[t][:, kt, c * P:(c + 1) * P],
                                proj_sb[t][:, kt, n * 512:(n + 1) * 512],
                                start=(kt == 0),
                                stop=(kt == K - 1),
                            )
                        half = ev[:, c, n * 512:(n + 1) * 512]
                        if n == 0:
                            nc.vector.tensor_copy(half, ps[:])
                        else:
                            nc.scalar.copy(half, ps[:])
                fc, rem = divmod(out_counts[t], P)
                groups = [(0, fc)] if fc <= 3 else [(0, fc // 2), (fc // 2, fc)]
                for ca, cb in groups:
                    if cb > ca:
                        nc.sync.dma_start(
                            outb[row0 + ca * P:row0 + cb * P, :]
                            .rearrange("(c p) n -> p c n", p=P),
                            ev[:, ca:cb, :],
                        )
                if rem:
                    nc.sync.dma_start(
                        outb[row0 + fc * P: row0 + fc * P + rem, :],
                        ev[:rem, fc, :],
                    )
                row0 += out_counts[t]

    nc.finalize()
    return nc


def _host_prep(inp):
    """Bucket tokens by table; dedup rows; sort by row; per-core counts."""
    flat = np.asarray(inp).reshape(-1).astype(np.int64)

    tbl = np.searchsorted(np.asarray(CUTS[1:]), flat, side="right")
    local = flat - np.asarray(CUTS)[tbl]

    positions = {}
    lidx = {}
    uniq = {}
    for t in range(4):
        pos = np.nonzero(tbl == t)[0]
        if not pos.size:
            continue
        rows = local[pos]
        u, inv = np.unique(rows, return_inverse=True)
        order = np.argsort(inv, kind="stable")   # sort tokens by table row
        positions[t] = pos[order]
        lidx[t] = inv[order].astype(np.int16)
        uniq[t] = u

    active = tuple(sorted(positions.keys()))
    out_counts = {}
    slot_counts = {}
    for t in active:
        n = len(positions[t])
        cg = -(-n // N_CORES)           # ceil(n / 8): rows per core
        out_counts[t] = cg
        slot_counts[t] = max(P, -(-cg // P) * P)
    return flat, active, positions, lidx, uniq, out_counts, slot_counts


def _idx_tensor(active, lidx, slot_counts, core):
    """Combined int16 [128, total_slots/16] tile for one core.

    Slot j of a group at [j%16, j//16] within the group's column window;
    pads read row 0.  HW's dma_gather on SWDGE queue q reads the indices
    from partitions 32q+16 .. 32q+31 while CoreSim reads 0-15 — write all
    five ranges so any queue assignment (and the sim) sees them.
    """
    total = sum(slot_counts[t] for t in active)
    arr = np.zeros((P, total // 16), np.int16)
    off = 0
    for t in active:
        li = lidx[t][core::N_CORES]
        j = np.arange(len(li))
        for base in (0, 16, 48, 80, 112):
            arr[base + j % 16, off // 16 + j // 16] = li
        off += slot_counts[t]
    return arr


def _prep_compact_tables(active, uniq, raw_tables, raw_projs):
    tables = {}
    projTs = {}
    for t in active:
        emb = raw_tables[t]
        sel = np.asarray(emb, dtype=np.float32)[uniq[t]]
        tb = np.zeros((len(uniq[t]), D_PAD[t]), BF16)
        tb[:, :emb.shape[1]] = sel.astype(BF16)
        tables[t] = tb
        proj = raw_projs[t]
        pt = np.zeros((D_PAD[t], D_OUT), np.float32)
        pt[:proj.shape[1], :] = (np.asarray(proj, np.float32) * EMB_SCALE).T
        projTs[t] = pt.astype(BF16)
    return tables, projTs


def kernel(inp, emb0, emb1, emb2, emb3, proj0, proj1, proj2, proj3):
    global LAST_RESULTS
    from concourse.bass_utils import run_bass_kernel_spmd

    flat, active, positions, lidx, uniq, out_counts, slot_counts = \
        _host_prep(inp)
    T = flat.shape[0]

    tables, projTs = _prep_compact_tables(
        active, uniq, (emb0, emb1, emb2, emb3), (proj0, proj1, proj2, proj3))
    tbl_rows = {t: tables[t].shape[0] for t in active}

    key = (active, tuple(slot_counts[t] for t in active),
           tuple(out_counts[t] for t in active),
           tuple(tbl_rows[t] for t in active))
    nc = _PROGRAM_CACHE.get(key)
    if nc is None:
        nc = _build_program(active, slot_counts, out_counts, tbl_rows)
        _PROGRAM_CACHE[key] = nc

    in_maps = []
    for k in range(N_CORES):
        m = {}
        for t in active:
            m[f"embt{t}"] = tables[t]
            m[f"projt{t}"] = projTs[t]
        m["idx"] = _idx_tensor(active, lidx, slot_counts, k)
        in_maps.append(m)

    trace = bool(os.environ.get("KERNEL_TRACE"))
    res = run_bass_kernel_spmd(nc, in_maps, core_ids=list(range(N_CORES)),
                               trace=trace)
    LAST_RESULTS = res

    out = np.empty((T, D_OUT), np.float32)
    bases = {}
    r0 = 0
    for t in active:
        bases[t] = r0
        r0 += out_counts[t]
    for k in range(N_CORES):
        ob = np.asarray(res.results[k]["outb"])
        for t in active:
            pos = positions[t][k::N_CORES]
            if pos.size:
                out[pos] = ob[bases[t]:bases[t] + len(pos)].astype(np.float32)

    return out.reshape(*np.asarray(inp).shape, D_OUT)


# revision 35
# speedup vs baseline: 1.1318x; 1.0128x over previous
"""Adaptive embedding (4-bucket) lookup + projection on 8 TRN2 NeuronCores.

Strategy: pure data-parallel over the 16384 tokens (no collectives).
  Host: bucket every token by its embedding table, deduplicate each table to
        the rows actually referenced (<= n_tokens distinct rows, so gather
        indices always fit int16), sort each bucket's tokens by row for HBM
        locality, and deal them evenly across the 8 cores so every core runs
        an identical-shape program.  Tables are pre-cast to bf16 with rows
        padded to a multiple of 128 elements; projections are pre-transposed,
        pre-scaled by sqrt(D) and zero-padded to match.
  Core: one dma_gather(transpose=True) per table pulls that bucket's
        embedding rows from HBM directly into d-on-partitions (matmul lhsT)
        layout; accumulating matmuls against the resident projT produce
        [128 tokens, 1024] in PSUM; DVE/ACT alternate evacuating to bf16 in
        SBUF; plain DMA stores the rows.
  Host: rows are scattered back to original token order and upcast to f32.
"""

import os
import sys

import numpy as np

for _p in ("/opt/trn_rl_repo",):
    if _p not in sys.path:
        sys.path.insert(0, _p)

import ml_dtypes

BF16 = ml_dtypes.bfloat16

N_TOKEN = 267735
CUTS = (0, 20000, 40000, 200000, N_TOKEN)
D_TBL = (1024, 256, 64, 16)
D_PAD = (1024, 256, 128, 128)
D_OUT = 1024
EMB_SCALE = float(D_OUT) ** 0.5
N_CORES = 8
P = 128

_PROGRAM_CACHE = {}
LAST_RESULTS = None  # BassKernelResults of the most recent run (for profiling)


def _build_program(active, slot_counts, out_counts, tbl_rows):
    """Build + compile the per-core Bass program.

    active: tuple of table ids with nonzero token count
    slot_counts / out_counts: per active table — gather slots (mult of 128)
        and output row count (identical on every core)
    tbl_rows: rows of each deduplicated bf16 table
    """
    import concourse.bacc as bacc
    import concourse.mybir as mybir
    import concourse.tile as tile

    dt = mybir.dt
    nc = bacc.Bacc("TRN2", target_bir_lowering=False, debug=False,
                   num_swdge_queues=4)

    embs = {
        t: nc.dram_tensor(f"embt{t}", [tbl_rows[t], D_PAD[t]], dt.bfloat16,
                          kind="ExternalInput")
        for t in active
    }
    projs = {
        t: nc.dram_tensor(f"projt{t}", [D_PAD[t], D_OUT], dt.bfloat16,
                          kind="ExternalInput")
        for t in active
    }
    total_slots = sum(slot_counts[t] for t in active)
    idx = nc.dram_tensor("idx", [P, total_slots // 16], dt.int16,
                         kind="ExternalInput")
    R = sum(out_counts[t] for t in active)
    outb = nc.dram_tensor("outb", [R, D_OUT], dt.bfloat16, kind="ExternalOutput")

    from concourse.library_config import mlp

    with tile.TileContext(nc) as tc:
        with (
            tc.tile_pool(name="const", bufs=1) as const_pool,
            tc.tile_pool(name="gath", bufs=1) as gath_pool,
            tc.tile_pool(name="evac", bufs=1) as evac_pool,
            tc.tile_pool(name="psum", bufs=8, space="PSUM") as psum_pool,
        ):
            # the Q7 mlp library (dma_gather) takes ~10us to land — start the
            # load as early as possible
            nc.gpsimd.load_library(mlp)

            # all token-index tiles in one small DMA, first in the queue
            idx_sb = const_pool.tile([P, total_slots // 16], dt.int16, tag="idx")
            nc.sync.dma_start(idx_sb[:], idx[:])

            # gathers: rows land transposed, [128, K, C] = emb^T K-tiles.
            # The Q7 gather kernel's index scratch caps num_idxs (~1K crashes
            # on HW) — split big gathers into <=MAX_GATHER column slices, and
            # spread pieces across the 4 SWDGE queues (distinct Q7 core
            # pairs) so their descriptor generation runs concurrently.
            MAX_GATHER = 768
            pieces = []  # (table, tile, col0, size, idx_off)
            gath_sb = {}
            off = 0
            for t in active:
                K = D_PAD[t] // P
                C = slot_counts[t]
                gt = gath_pool.tile([P, K, C], dt.bfloat16, tag=f"g{t}")
                n_piece = -(-C // MAX_GATHER)
                piece = -(-(C // P) // n_piece) * P
                assert n_piece == 1 or K == 1
                for c0 in range(0, C, piece):
                    cs = min(piece, C - c0)
                    pieces.append((t, gt, c0, cs, off + c0, n_piece > 1))
                gath_sb[t] = gt
                off += C
            # schedule: table 0 first (its matmuls gate the PE start; the
            # first-dispatched gather begins ~2us before the rest), then big
            # pieces, round-robin over the 4 queues.
            # NOTE: overflow gathers (beyond one per queue) must cycle back
            # to queue 0 — a second gather issued on queue 3 while others
            # are in flight corrupts lanes 4/6/7 of concurrent gathers
            # (HW-reproduced; see probe5 experiments).
            pieces.sort(key=lambda p: (p[0] != 0, -p[3]))
            g0_inst = None
            for i, (t, gt, c0, cs, ioff, sliced) in enumerate(pieces):
                q = i % 4
                gi = nc.gpsimd.dma_gather(
                    gt[:, :, c0:c0 + cs] if sliced else gt[:],
                    embs[t][:, :],
                    idx_sb[:, ioff // 16:(ioff + cs) // 16],
                    cs,
                    cs,
                    D_PAD[t],
                    transpose=True,
                    queue_num=q,
                )
                if g0_inst is None:
                    g0_inst = gi

            # resident projections: [Dp, 1024] -> [128, K, 1024].
            # Split each into per-K-tile DMAs so the first matmuls only wait
            # for the K-tiles they read.  Hold them until the first gather's
            # descriptor generation completes: their 2.75MB of HBM traffic
            # otherwise contends with the Q7 library-image load and delays
            # the gathers by up to ~8us on some cores.
            import concourse.bass as bass

            proj_sb = {}
            for t in active:
                K = D_PAD[t] // P
                pt = const_pool.tile([P, K, D_OUT], dt.bfloat16, tag=f"proj{t}")
                src = projs[t][:, :].rearrange("(k p) n -> p k n", p=P)
                for k in range(K):
                    pi = nc.sync.dma_start(pt[:, k, :], src[:, k, :])
                    bass._add_dep_helper(
                        pi.ins, g0_inst.ins, sync=True,
                        reason="projT loads after lib+first gather")
                proj_sb[t] = pt

            # per 128-token chunk: accumulate over K into PSUM; as soon as
            # each 512-wide bank's chain completes, evacuate that half on
            # DVE / ACT (one engine per half, in parallel); store each
            # table with 1-2 big DMAs from a per-table staging tile
            row0 = 0
            for t in active:
                K = D_PAD[t] // P
                n_c = -(-out_counts[t] // P)
                ev = evac_pool.tile([P, n_c, D_OUT], dt.bfloat16, tag=f"ev{t}")
                for c in range(n_c):
                    for n in range(2):
                        ps = psum_pool.tile([P, 512], dt.float32, tag="ps")
                        for kt in range(K):
                            nc.tensor.matmul(
                                ps[:],
                                gath_sb[t][:, kt, c * P:(c + 1) * P],
                                proj_sb[t][:, kt, n * 512:(n + 1) * 512],
                                start=(kt == 0),
                                stop=(kt == K - 1),
                            )
                        half = ev[:, c, n * 512:(n + 1) * 512]
                        if n == 0:
                            nc.vector.tensor_copy(half, ps[:])
                        else:
                            nc.scalar.copy(half, ps[:])
                fc, rem = divmod(out_counts[t], P)
                groups = [(0, fc)] if fc <= 3 else [(0, fc // 2), (fc // 2, fc)]
                for ca, cb in groups:
                    if cb > ca:
                        nc.sync.dma_start(
                            outb[row0 + ca * P:row0 + cb * P, :]
                            .rearrange("(c p) n -> p c n", p=P),
                            ev[:, ca:cb, :],
                        )
                if rem:
                    nc.sync.dma_start(
                        outb[row0 + fc * P: row0 + fc * P + rem, :],
                        ev[:rem, fc, :],
                    )
                row0 += out_counts[t]

    nc.finalize()
    return nc


def _host_prep(inp):
    """Bucket tokens by table; dedup rows; sort by row; per-core counts."""
    flat = np.asarray(inp).reshape(-1).astype(np.int64)

    tbl = np.searchsorted(np.asarray(CUTS[1:]), flat, side="right")
    local = flat - np.asarray(CUTS)[tbl]

    positions = {}
    lidx = {}
    uniq = {}
    for t in range(4):
        pos = np.nonzero(tbl == t)[0]
        if not pos.size:
            continue
        rows = local[pos]
        u, inv = np.unique(rows, return_inverse=True)
        order = np.argsort(inv, kind="stable")   # sort tokens by table row
        positions[t] = pos[order]
        lidx[t] = inv[order].astype(np.int16)
        uniq[t] = u

    active = tuple(sorted(positions.keys()))
    out_counts = {}
    slot_counts = {}
    for t in active:
        n = len(positions[t])
        cg = -(-n // N_CORES)           # ceil(n / 8): rows per core
        out_counts[t] = cg
        slot_counts[t] = max(P, -(-cg // P) * P)
    return flat, active, positions, lidx, uniq, out_counts, slot_counts


def _idx_tensor(active, lidx, slot_counts, core):
    """Combined int16 [128, total_slots/16] tile for one core.

    Slot j of a group at [j%16, j//16] within the group's column window;
    pads read row 0.  HW's dma_gather on SWDGE queue q reads the indices
    from partitions 32q+16 .. 32q+31 while CoreSim reads 0-15 — write all
    five ranges so any queue assignment (and the sim) sees them.
    """
    total = sum(slot_counts[t] for t in active)
    arr = np.zeros((P, total // 16), np.int16)
    off = 0
    for t in active:
        li = lidx[t][core::N_CORES]
        j = np.arange(len(li))
        for base in (0, 16, 48, 80, 112):
            arr[base + j % 16, off // 16 + j // 16] = li
        off += slot_counts[t]
    return arr


def _prep_compact_tables(active, uniq, raw_tables, raw_projs):
    tables = {}
    projTs = {}
    for t in active:
        emb = raw_tables[t]
        sel = np.asarray(emb, dtype=np.float32)[uniq[t]]
        tb = np.zeros((len(uniq[t]), D_PAD[t]), BF16)
        tb[:, :emb.shape[1]] = sel.astype(BF16)
        tables[t] = tb
        proj = raw_projs[t]
        pt = np.zeros((D_PAD[t], D_OUT), np.float32)
        pt[:proj.shape[1], :] = (np.asarray(proj, np.float32) * EMB_SCALE).T
        projTs[t] = pt.astype(BF16)
    return tables, projTs


def kernel(inp, emb0, emb1, emb2, emb3, proj0, proj1, proj2, proj3):
    global LAST_RESULTS
    from concourse.bass_utils import run_bass_kernel_spmd

    flat, active, positions, lidx, uniq, out_counts, slot_counts = \
        _host_prep(inp)
    T = flat.shape[0]

    tables, projTs = _prep_compact_tables(
        active, uniq, (emb0, emb1, emb2, emb3), (proj0, proj1, proj2, proj3))
    tbl_rows = {t: tables[t].shape[0] for t in active}

    key = (active, tuple(slot_counts[t] for t in active),
           tuple(out_counts[t] for t in active),
           tuple(tbl_rows[t] for t in active))
    nc = _PROGRAM_CACHE.get(key)
    if nc is None:
        nc = _build_program(active, slot_counts, out_counts, tbl_rows)
        _PROGRAM_CACHE[key] = nc

    in_maps = []
    for k in range(N_CORES):
        m = {}
        for t in active:
            m[f"embt{t}"] = tables[t]
            m[f"projt{t}"] = projTs[t]
        m["idx"] = _idx_tensor(active, lidx, slot_counts, k)
        in_maps.append(m)

    trace = bool(os.environ.get("KERNEL_TRACE"))
    res = run_bass_kernel_spmd(nc, in_maps, core_ids=list(range(N_CORES)),
                               trace=trace)
    LAST_RESULTS = res

    out = np.empty((T, D_OUT), np.float32)
    bases = {}
    r0 = 0
    for t in active:
        bases[t] = r0
        r0 += out_counts[t]
    for k in range(N_CORES):
        ob = np.asarray(res.results[k]["outb"])
        for t in active:
            pos = positions[t][k::N_CORES]
            if pos.size:
                out[pos] = ob[bases[t]:bases[t] + len(pos)].astype(np.float32)

    return out.reshape(*np.asarray(inp).shape, D_OUT)
